# revision 5
# baseline (speedup 1.0000x reference)
"""VQ codebook cosine-similarity softmax kernel for Trainium2 (8 NeuronCores).

Computes softmax(cos_sim(batch, centroids)) for batch [131072, 1024] f32 and
centroids [256, 1024] f32, data-parallel over the batch dim across 8 cores.

Per-core pipeline (16384 rows):
  - SWDGE cast-DMA loads x tiles HBM f32 -> SBUF fp16 (halves SBUF traffic,
    enables full-rate fp16 matmuls; fp32 accumulation keeps rel err ~1e-4)
  - PE transposes each [128,128] fp16 block (x must have D on partitions for
    the matmul contraction), DVE copies PSUM->SBUF
  - PE matmul: weights = xT block [128d,128n], moving = cnT [128d,256k],
    accumulating over 8 d-chunks into PSUM f32 [128n, 256k]
  - row norms: ACT Square+accum over row-major fp16 x; 1/sqrt via
    exp(-0.5*ln(s)) (same ACT table set as Exp) + one DVE Newton step
  - softmax: logits = cos in [-1,1] so no max-subtraction needed;
    ACT Exp(scale=1/||x||) with accum_out giving the denominator,
    DVE reciprocal + tensor_scalar multiply
"""

import sys

if "/opt/trn_rl_repo" not in sys.path:
    sys.path.insert(0, "/opt/trn_rl_repo")

import numpy as np

N, D, K = 131072, 1024, 256
NCORES = 8
NPC = N // NCORES  # rows per core
P = 128  # partitions / tile rows
XB = 4  # row-tiles per load/store DMA batch
G = 16  # row-tiles per norm group (batched ln/exp)


def build_bass(npc=NPC):
    """Build the single-core SPMD program; every core runs this with its own
    x shard. Returns the compiled Bacc object."""
    from contextlib import ExitStack

    import concourse.bacc as bacc
    import concourse.mybir as mybir
    import concourse.tile as tile
    from concourse.masks import make_identity

    dt = mybir.dt
    AFT = mybir.ActivationFunctionType
    Alu = mybir.AluOpType

    nt = npc // P  # row tiles
    assert npc % (P * XB) == 0
    ngroups = (nt + G - 1) // G

    nc = bacc.Bacc(
        "TRN2", target_bir_lowering=False, debug=False, num_devices=NCORES
    )
    x_d = nc.dram_tensor("x", [npc, D], dt.float32, kind="ExternalInput")
    c_d = nc.dram_tensor("c", [K, D], dt.float32, kind="ExternalInput")
    o_d = nc.dram_tensor("o", [npc, K], dt.float32, kind="ExternalOutput")

    ND = D // P  # d-chunks (8)

    with tile.TileContext(nc) as tc, ExitStack() as ctx:
        const = ctx.enter_context(tc.tile_pool(name="const", bufs=1))
        ident = const.tile([P, P], dt.float16)
        make_identity(nc, ident[:])

        # cnT: [128 (d within chunk), ND * K] fp16; chunk b at cols [K*b, K*b+K)
        cnT = const.tile([P, ND * K], dt.float16)
        # per-tile squared row norms, one column per row-tile
        norm2cols = const.tile([P, max(nt, 1)], dt.float32)

        # ---- centroid prep (one-time, ~1MB) ----
        cprep = ctx.enter_context(tc.tile_pool(name="cprep", bufs=2))
        cpsum = ctx.enter_context(tc.tile_pool(name="cpsum", bufs=2, space="PSUM"))
        for h in range(K // P):  # 2 halves of the K=256 centroids
            c32 = cprep.tile([P, D], dt.float32, tag="c32")
            nc.sync.dma_start(c32[:], c_d.ap()[P * h : P * (h + 1), :])
            csq = cprep.tile([P, D], dt.float32, tag="csq")
            cn2 = cprep.tile([P, 1], dt.float32, tag="cn2")
            nc.scalar.activation(csq[:], c32[:], AFT.Square, accum_out=cn2[:])
            # rnorm = exp(-0.5 * ln(norm2)), then one Newton step in f32
            cln = cprep.tile([P, 1], dt.float32, tag="cln")
            nc.scalar.activation(cln[:], cn2[:], AFT.Ln)
            cy0 = cprep.tile([P, 1], dt.float32, tag="cy0")
            nc.scalar.activation(cy0[:], cln[:], AFT.Exp, scale=-0.5)
            ct1 = cprep.tile([P, 1], dt.float32, tag="ct1")
            nc.vector.tensor_tensor(ct1[:], cy0[:], cy0[:], Alu.mult)
            nc.vector.tensor_tensor(ct1[:], ct1[:], cn2[:], Alu.mult)
            nc.vector.tensor_scalar(ct1[:], ct1[:], -0.5, 1.5, Alu.mult, Alu.add)
            nc.vector.tensor_tensor(cy0[:], cy0[:], ct1[:], Alu.mult)
            cn16 = cprep.tile([P, D], dt.float16, tag="cn16")
            nc.vector.tensor_scalar_mul(cn16[:], c32[:], cy0[:])
            for b in range(ND):
                pt = cpsum.tile([P, P], dt.float16, tag="ct_ps")
                nc.tensor.transpose(pt[:], cn16[:, P * b : P * (b + 1)], ident[:])
                nc.vector.tensor_copy(
                    cnT[:, K * b + P * h : K * b + P * h + P], pt[:]
                )

        # ---- main loop ----
        x16_pool = ctx.enter_context(tc.tile_pool(name="x16", bufs=2 * G // XB))
        xt_pool = ctx.enter_context(tc.tile_pool(name="xt", bufs=3))
        sq_pool = ctx.enter_context(tc.tile_pool(name="sq", bufs=2))
        e_pool = ctx.enter_context(tc.tile_pool(name="e", bufs=3))
        pm_pool = ctx.enter_context(tc.tile_pool(name="pm", bufs=3))
        nrm_pool = ctx.enter_context(tc.tile_pool(name="nrm", bufs=2 * ngroups))
        den_pool = ctx.enter_context(tc.tile_pool(name="den", bufs=4))
        tps_pool = ctx.enter_context(
            tc.tile_pool(name="tps", bufs=2, space="PSUM")
        )
        sps_pool = ctx.enter_context(
            tc.tile_pool(name="sps", bufs=3, space="PSUM")
        )

        for g in range(ngroups):
            t0 = g * G
            t1 = min(t0 + G, nt)
            gtiles = range(t0, t1)
            # 1) cast-loads (XB row-tiles per DMA) + 2) row-norm squares
            xmacs = {}
            for tm in range(t0 // XB, (t1 + XB - 1) // XB):
                xm = x16_pool.tile([P, XB * D], dt.float16, tag="xm")
                src = x_d.ap()[P * XB * tm : P * XB * (tm + 1), :].rearrange(
                    "(s p) d -> p s d", s=XB
                )
                nc.gpsimd.dma_start(
                    xm[:].rearrange("p (s d) -> p s d", s=XB), src
                )
                xmacs[tm] = xm
            for t in gtiles:
                xm = xmacs[t // XB]
                xs = xm[:, D * (t % XB) : D * (t % XB + 1)]
                sq = sq_pool.tile([P, D], dt.float16, tag="sq")
                nc.scalar.activation(
                    sq[:], xs, AFT.Square, accum_out=norm2cols[:, t : t + 1]
                )
            # 3) batched rnorm = exp(-0.5 ln s) + Newton step
            gw = t1 - t0
            n2g = norm2cols[:, t0:t1]
            lng = nrm_pool.tile([P, G], dt.float32, tag="lng")
            nc.scalar.activation(lng[:, :gw], n2g, AFT.Ln)
            rng = nrm_pool.tile([P, G], dt.float32, tag="rng")
            nc.scalar.activation(rng[:, :gw], lng[:, :gw], AFT.Exp, scale=-0.5)
            nt1 = nrm_pool.tile([P, G], dt.float32, tag="nt1")
            nc.vector.tensor_tensor(nt1[:, :gw], rng[:, :gw], rng[:, :gw], Alu.mult)
            nc.vector.tensor_tensor(nt1[:, :gw], nt1[:, :gw], n2g, Alu.mult)
            nc.vector.tensor_scalar(
                nt1[:, :gw], nt1[:, :gw], -0.5, 1.5, Alu.mult, Alu.add
            )
            nc.vector.tensor_tensor(rng[:, :gw], rng[:, :gw], nt1[:, :gw], Alu.mult)
            # 4) per-tile transpose -> matmul -> softmax
            pmacs = {}
            for t in gtiles:
                xm = xmacs[t // XB]
                xs = xm[:, D * (t % XB) : D * (t % XB + 1)]
                tps = tps_pool.tile([P, D], dt.float16, tag="tps")
                for b in range(ND):
                    nc.tensor.transpose(
                        tps[:, P * b : P * (b + 1)],
                        xs[:, P * b : P * (b + 1)],
                        ident[:],
                    )
                xt = xt_pool.tile([P, D], dt.float16, tag="xt")
                nc.vector.tensor_copy(xt[:], tps[:])
                sps = sps_pool.tile([P, K], dt.float32, tag="sps")
                for b in range(ND):
                    nc.tensor.matmul(
                        sps[:],
                        xt[:, P * b : P * (b + 1)],
                        cnT[:, K * b : K * (b + 1)],
                        start=(b == 0),
                        stop=(b == ND - 1),
                    )
                e = e_pool.tile([P, K], dt.float32, tag="e")
                den = den_pool.tile([P, 1], dt.float32, tag="den")
                j = t - t0
                nc.scalar.activation(
                    e[:], sps[:], AFT.Exp,
                    scale=rng[:, j : j + 1], accum_out=den[:],
                )
                rden = den_pool.tile([P, 1], dt.float32, tag="rden")
                nc.vector.reciprocal(rden[:], den[:])
                tm = t // XB
                if tm not in pmacs:
                    pmac = pm_pool.tile([P, XB * K], dt.float32, tag="pmac")
                    pmacs[tm] = pmac
                pm = pmacs[tm]
                nc.vector.tensor_scalar_mul(
                    pm[:, K * (t % XB) : K * (t % XB + 1)], e[:], rden[:]
                )
                if t % XB == XB - 1:
                    dst = o_d.ap()[
                        P * XB * tm : P * XB * (tm + 1), :
                    ].rearrange("(s p) k -> p s k", s=XB)
                    nc.sync.dma_start(
                        dst, pm[:].rearrange("p (s k) -> p s k", s=XB)
                    )

    nc.compile()
    return nc


_cache = {}


def _get_nc(npc=NPC):
    if npc not in _cache:
        _cache[npc] = build_bass(npc)
    return _cache[npc]


def kernel(batch: np.ndarray, centroids: np.ndarray) -> np.ndarray:
    from concourse.bass_utils import run_bass_kernel_spmd

    assert batch.shape == (N, D) and centroids.shape == (K, D)
    batch = np.ascontiguousarray(batch, dtype=np.float32)
    centroids = np.ascontiguousarray(centroids, dtype=np.float32)

    nc = _get_nc()
    in_maps = [
        {"x": batch[i * NPC : (i + 1) * NPC], "c": centroids}
        for i in range(NCORES)
    ]
    res = run_bass_kernel_spmd(nc, in_maps, core_ids=list(range(NCORES)))
    return np.concatenate([res.results[i]["o"] for i in range(NCORES)], axis=0)
